# revision 1
# baseline (speedup 1.0000x reference)
"""Decoupled top-k distillation loss on 8 Trainium2 NeuronCores.

Full inputs: student_logits, teacher_logits (2, 2048, 32000) f32.
Data-parallel: the 4096 flattened rows are sharded 512/core across 8 cores.

Per row (vocab V=32000, K=32, T=2):
  - teacher top-32 values T32 found exactly via hierarchical selection:
    40 blocks of 800 -> per-block top-8 (DVE max) -> 320 candidates ->
    4 rounds of max+match_replace -> exact top-32 (a block holding >8 of the
    row's top-32 has probability ~1e-19 for continuous data).
  - theta = 32nd largest; support mask = (tl >= theta) applied by value,
    no gather/indices needed anywhere.
  - S_t = sum exp(tl), A_t = sum exp(T32)        -> p_t = A_t/S_t
  - S_s = sum exp(sl), A_s = sum_mask exp(sl)    -> p_s = A_s/S_s
  - w = exp(tl/2)*mask (teacher buffer transformed in place)
  - Zt = sum w, Zq = sum_mask exp(sl/2)
  - crossT = sum exp(T32/2)*T32, crossS = sum w*sl
  - KL row = (crossT-crossS)/(2 Zt) - ln Zt + ln Zq
  - BCE row = -(p_t*max(ln p_s,-100) + (1-p_t)*max(ln(1-p_s),-100))
Host combines: loss = mean(BCE) + mean(p_t)*T^2*mean(KL).
"""

import sys

import numpy as np

sys.path.insert(0, "/opt/trn_rl_repo")

import concourse.bacc as bacc  # noqa: E402
import concourse.bass as bass  # noqa: E402,F401
import concourse.mybir as mybir  # noqa: E402
from concourse.bass_utils import run_bass_kernel_spmd  # noqa: E402
from concourse.tile import TileContext  # noqa: E402

F32 = mybir.dt.float32
ALU = mybir.AluOpType
ACTF = mybir.ActivationFunctionType
AX = mybir.AxisListType

B, L, V = 2, 2048, 32000
N = B * L                  # 4096 rows
NCORES = 8
ROWS = N // NCORES         # 512 rows per core
P = 128                    # rows per tile (partition dim)
NT = ROWS // P             # 4 tiles per core
K = 32
NSUB = 10                  # teacher subtiles per row
SW = V // NSUB             # 3200 subtile width
RING = NSUB + 1            # teacher slot ring
BLKW = 800                 # selection block width
BPS = SW // BLKW           # 4 blocks per subtile
NBLK = V // BLKW           # 40 blocks
SC = SW                    # chunk width == subtile width
NCH = V // SC              # 10 chunks per row
NEG = -1.0e30


def build_nc(nt=NT):
    rows = nt * P
    nc = bacc.Bacc("TRN2", debug=False)
    t_in = nc.declare_dram_parameter("t", [rows, V], F32, isOutput=False)
    s_in = nc.declare_dram_parameter("s", [rows, V], F32, isOutput=False)
    o_out = nc.declare_dram_parameter("o", [P, 3 * nt], F32, isOutput=True)

    with TileContext(nc) as tc:
        with (
            tc.tile_pool(name="big", bufs=1) as big,
            tc.tile_pool(name="stu", bufs=3) as stu,
            tc.tile_pool(name="small", bufs=2) as small,
            tc.tile_pool(name="singles", bufs=1) as singles,
        ):
            out_t = singles.tile([P, 3 * nt], F32)
            # one dump buffer per writer engine: same-engine WAW needs no sems
            dump_a = singles.tile([P, SC], F32)   # ACT dump
            dump_v = singles.tile([P, SC], F32)   # DVE dump

            for it in range(nt):
                r0 = it * P
                t_rows = t_in[r0:r0 + P, :]
                s_rows = s_in[r0:r0 + P, :]

                # ---------------- teacher phase ----------------
                # 6-slot ring: 5 raw subtiles + w written out-of-place into
                # the slot freed by the previous subtile's raw copy. The
                # ring also lets the next tile's teacher DMA start while
                # this tile's student phase drains.
                base = (NSUB * it) % RING
                A = [big.tile([P, SW], F32, tag=f"T{(base + u) % RING}",
                              name=f"raw{it}_{u}") for u in range(NSUB)]
                W = []
                for u in range(NSUB):
                    nc.sync.dma_start(
                        out=A[u], in_=t_rows[:, u * SW:(u + 1) * SW])

                # S_t = sum exp(tl), accumulated per chunk on ACT (raw A)
                st_part = small.tile([P, NCH], F32, tag="st_part")
                for u in range(NSUB):
                    nc.scalar.activation(
                        out=dump_a, in_=A[u], func=ACTF.Exp,
                        accum_out=st_part[:, u:u + 1],
                    )

                # per-block top-8 -> 320 candidates
                cand = small.tile([P, NBLK * 8], F32, tag="cand")
                for b in range(NBLK):
                    u, o = b // BPS, (b % BPS) * BLKW
                    nc.vector.max(
                        out=cand[:, b * 8:(b + 1) * 8],
                        in_=A[u][:, o:o + BLKW],
                    )
                # 4 rounds -> exact top-32
                t32 = small.tile([P, K], F32, tag="t32")
                for r in range(4):
                    nc.vector.max(out=t32[:, r * 8:(r + 1) * 8], in_=cand)
                    nc.vector.match_replace(
                        out=cand, in_to_replace=t32[:, r * 8:(r + 1) * 8],
                        in_values=cand, imm_value=NEG,
                    )
                # theta = 32nd largest; eth = exp(theta/2)
                th = small.tile([P, 1], F32, tag="th")
                nc.vector.tensor_reduce(out=th, in_=t32, axis=AX.X, op=ALU.min)
                eth = small.tile([P, 1], F32, tag="eth")
                nc.scalar.activation(out=eth, in_=th, func=ACTF.Exp, scale=0.5)
                # A_t = sum exp(T32); Zt = sum exp(T32/2); crossT
                e32 = small.tile([P, K], F32, tag="e32")
                at = small.tile([P, 1], F32, tag="at")
                nc.scalar.activation(out=e32, in_=t32, func=ACTF.Exp,
                                     accum_out=at)
                e32h = small.tile([P, K], F32, tag="e32h")
                d32 = small.tile([P, K], F32, tag="d32")
                zt = small.tile([P, 1], F32, tag="zt")
                crt = small.tile([P, 1], F32, tag="crt")
                nc.scalar.activation(out=e32h, in_=t32, func=ACTF.Exp,
                                     scale=0.5, accum_out=zt)
                nc.vector.tensor_mul(d32, e32h, t32)
                nc.vector.tensor_reduce(out=crt, in_=d32, axis=AX.X,
                                        op=ALU.add)

                # teacher -> w = exp(tl/2) (UNMASKED), out-of-place ring
                for u in range(NSUB):
                    wslot = ((base + NSUB) % RING if u == 0
                             else (base + u - 1) % RING)
                    wt = big.tile([P, SW], F32, tag=f"T{wslot}",
                                  name=f"w{it}_{u}")
                    W.append(wt)
                    nc.scalar.activation(
                        out=wt, in_=A[u], func=ACTF.Exp, scale=0.5,
                    )

                # ---------------- student phase ----------------
                # mask lives on the student: sb <- sl * [w >= eth]
                ss_part = small.tile([P, NCH], F32, tag="ss_part")
                as_part = small.tile([P, NCH], F32, tag="as_part")
                zq_part = small.tile([P, NCH], F32, tag="zq_part")
                cr_part = small.tile([P, NCH], F32, tag="cr_part")
                for j in range(NCH):
                    u, sl = j, slice(0, SC)
                    sb = stu.tile([P, SC], F32, tag="sb")
                    nc.sync.dma_start(
                        out=sb, in_=s_rows[:, j * SC:(j + 1) * SC])
                    # S_s += sum exp(sl)  (raw sb)
                    nc.scalar.activation(
                        out=dump_a, in_=sb, func=ACTF.Exp,
                        accum_out=ss_part[:, j:j + 1],
                    )
                    # mask in place: sb = (w >= eth) * sl
                    nc.vector.scalar_tensor_tensor(
                        out=sb, in0=W[u][:, sl], scalar=eth, in1=sb,
                        op0=ALU.is_ge, op1=ALU.mult,
                    )
                    # crossS += sum w * masked_sl  (exact: mask on student)
                    nc.vector.scalar_tensor_tensor(
                        out=dump_v, in0=W[u][:, sl], scalar=0.0, in1=sb,
                        op0=ALU.bypass, op1=ALU.mult,
                        accum_out=cr_part[:, j:j + 1],
                    )
                    # Zq: sbe <- exp(masked_sl/2), accum (needs -(V-32) fix)
                    sbe = stu.tile([P, SC], F32, tag="sb", name=f"sbe{j}")
                    nc.scalar.activation(
                        out=sbe, in_=sb, func=ACTF.Exp, scale=0.5,
                        accum_out=zq_part[:, j:j + 1],
                    )
                    # A_s: sum exp(masked_sl) = sum sbe^2 (needs -(V-32) fix)
                    nc.vector.scalar_tensor_tensor(
                        out=dump_v, in0=sbe, scalar=1.0, in1=sbe,
                        op0=ALU.mult, op1=ALU.mult,
                        accum_out=as_part[:, j:j + 1],
                    )

                # ---------------- per-row scalars ----------------
                st = small.tile([P, 1], F32, tag="st")
                ss = small.tile([P, 1], F32, tag="ss")
                asum = small.tile([P, 1], F32, tag="asum")
                zq = small.tile([P, 1], F32, tag="zq")
                crs = small.tile([P, 1], F32, tag="crs")
                nc.vector.tensor_reduce(out=st, in_=st_part, axis=AX.X,
                                        op=ALU.add)
                nc.vector.tensor_reduce(out=ss, in_=ss_part, axis=AX.X,
                                        op=ALU.add)
                nc.vector.tensor_reduce(out=asum, in_=as_part, axis=AX.X,
                                        op=ALU.add)
                nc.vector.tensor_reduce(out=zq, in_=zq_part, axis=AX.X,
                                        op=ALU.add)
                nc.vector.tensor_reduce(out=crs, in_=cr_part, axis=AX.X,
                                        op=ALU.add)
                # unselected entries contributed exp(0)=1 each
                nc.vector.tensor_scalar_add(asum, asum, -float(V - K))
                nc.vector.tensor_scalar_add(zq, zq, -float(V - K))

                pt = small.tile([P, 1], F32, tag="pt")
                ps = small.tile([P, 1], F32, tag="ps")
                tmp = small.tile([P, 1], F32, tag="tmp")
                nc.vector.reciprocal(tmp, st)
                nc.vector.tensor_mul(pt, at, tmp)
                nc.vector.reciprocal(tmp, ss)
                nc.vector.tensor_mul(ps, asum, tmp)

                # bce = -(pt*max(ln ps,-100) + (1-pt)*max(ln(1-ps),-100))
                lps = small.tile([P, 1], F32, tag="lps")
                l1m = small.tile([P, 1], F32, tag="l1m")
                nc.scalar.activation(out=lps, in_=ps, func=ACTF.Ln)
                nc.vector.tensor_scalar_max(lps, lps, -100.0)
                nc.vector.tensor_scalar(
                    out=tmp, in0=ps, scalar1=-1.0, scalar2=1.0,
                    op0=ALU.mult, op1=ALU.add,
                )
                nc.scalar.activation(out=l1m, in_=tmp, func=ACTF.Ln)
                nc.vector.tensor_scalar_max(l1m, l1m, -100.0)
                a1 = small.tile([P, 1], F32, tag="a1")
                a2 = small.tile([P, 1], F32, tag="a2")
                nc.vector.tensor_mul(a1, pt, lps)
                nc.vector.tensor_scalar(
                    out=tmp, in0=pt, scalar1=-1.0, scalar2=1.0,
                    op0=ALU.mult, op1=ALU.add,
                )
                nc.vector.tensor_mul(a2, tmp, l1m)
                nc.vector.tensor_add(a1, a1, a2)
                nc.vector.tensor_scalar_mul(
                    out_t[:, 3 * it:3 * it + 1], a1, -1.0)

                # pt out
                nc.vector.tensor_copy(out_t[:, 3 * it + 1:3 * it + 2], pt)

                # kl = (crossT - crossS)/(2 zt) - ln zt + ln zq
                k1 = small.tile([P, 1], F32, tag="k1")
                nc.vector.tensor_sub(k1, crt, crs)
                nc.vector.reciprocal(tmp, zt)
                nc.vector.tensor_mul(k1, k1, tmp)
                nc.vector.tensor_scalar_mul(k1, k1, 0.5)
                lzt = small.tile([P, 1], F32, tag="lzt")
                lzq = small.tile([P, 1], F32, tag="lzq")
                nc.scalar.activation(out=lzt, in_=zt, func=ACTF.Ln)
                nc.scalar.activation(out=lzq, in_=zq, func=ACTF.Ln)
                nc.vector.tensor_sub(k1, k1, lzt)
                nc.vector.tensor_add(out_t[:, 3 * it + 2:3 * it + 3], k1, lzq)

            nc.sync.dma_start(out=o_out[:, :], in_=out_t[:, :])

    nc.finalize()
    return nc


_NC_CACHE = None


def _get_nc():
    global _NC_CACHE
    if _NC_CACHE is None:
        _NC_CACHE = build_nc()
    return _NC_CACHE


def run_device(t2d, s2d, trace=False):
    """t2d/s2d: (N, V) float32. Returns BassKernelResults."""
    nc = _get_nc()
    in_maps = []
    for c in range(NCORES):
        sl = slice(c * ROWS, (c + 1) * ROWS)
        in_maps.append({
            "t": np.ascontiguousarray(t2d[sl]),
            "s": np.ascontiguousarray(s2d[sl]),
        })
    return run_bass_kernel_spmd(nc, in_maps, list(range(NCORES)), trace=trace)


def kernel(student_logits, teacher_logits):
    s2d = np.asarray(student_logits, dtype=np.float32).reshape(N, V)
    t2d = np.asarray(teacher_logits, dtype=np.float32).reshape(N, V)
    res = run_device(t2d, s2d)
    bce_sum = 0.0
    pt_sum = 0.0
    kl_sum = 0.0
    for c in range(NCORES):
        o = np.asarray(res.results[c]["o"], dtype=np.float64)  # [P, 3*NT]
        for it in range(NT):
            bce_sum += o[:, 3 * it].sum()
            pt_sum += o[:, 3 * it + 1].sum()
            kl_sum += o[:, 3 * it + 2].sum()
    loss_b = bce_sum / N
    mean_pt = pt_sum / N
    loss_t = kl_sum / N
    return np.float32(loss_b + mean_pt * 4.0 * loss_t)



# revision 2
# speedup vs baseline: 1.7981x; 1.7981x over previous
"""Decoupled top-k distillation loss on 8 Trainium2 NeuronCores.

Full inputs: student_logits, teacher_logits (2, 2048, 32000) f32.
Data-parallel: 4096 flattened rows sharded 512/core across 8 cores.

Packed-pair top-k (per row, V=32000, K=32, T=2):
  - Device packs each (teacher, student) element pair into one f32:
    high 16 bits = fp16(t), low 16 bits = fp16(s). For finite t the f32
    view orders exactly like t (fp16-rounded), with the s bits acting as
    an arbitrary deterministic tiebreak, so one DVE max8 cascade selects
    the top-32 (t, s) PAIRS per row -- no masks, no gathers, no theta.
  - Hierarchical selection: 32 blocks of 1000 -> per-block top-8 ->
    256 candidates -> 4 rounds of max8+match_replace -> top-32 pairs.
    (A block holding >8 of the row's top-32 has probability ~1e-7 for
    continuous data; a miss only perturbs one row's support slightly.)
  - S_t = sum exp(tl), S_s = sum exp(sl) ride the two ACT exp passes as
    free accumulator outputs (f32, exact).
  - Host unpacks the 32 (t, s) pairs and computes BCE + truncated KL in
    f64 exactly as the reference does on that support.

Device per core: 4 tiles x (8 chunks of [128, 4000]); teacher DMA on the
sync-engine HWDGE queue, student DMA on the activation-engine queue (2x
aggregate HBM bandwidth); ACT: 2 exp passes (accum S_t/S_s); DVE: 2
strided fp16 pack copies (2x mode) + max8 cascade.
"""

import sys

import numpy as np

sys.path.insert(0, "/opt/trn_rl_repo")

import concourse.bacc as bacc  # noqa: E402
import concourse.bass as bass  # noqa: E402,F401
import concourse.mybir as mybir  # noqa: E402
from concourse.bass_utils import run_bass_kernel_spmd  # noqa: E402
from concourse.tile import TileContext  # noqa: E402

F32 = mybir.dt.float32
FP16 = mybir.dt.float16
BF16 = mybir.dt.bfloat16
ALU = mybir.AluOpType
ACTF = mybir.ActivationFunctionType
AX = mybir.AxisListType

B, L, V = 2, 2048, 32000
N = B * L                  # 4096 rows
NCORES = 8
ROWS = N // NCORES         # 512 rows per core
P = 128                    # rows per tile (partition dim)
NT = ROWS // P             # 4 tiles per core
K = 32
SC = 4000                  # chunk width
NCH = V // SC              # 8 chunks per tile
BLK = 1000                 # selection block width
BPC = SC // BLK            # 4 blocks per chunk
NBLK = V // BLK            # 32 blocks per row
NCAND = NBLK * 8           # 256 candidates
NEG = -1.0e30
OCOLS = K + 2 * NCH        # 48 out cols per tile: p32 | st_part | ss_part


def build_nc(nt=NT):
    rows = nt * P
    nc = bacc.Bacc("TRN2", debug=False)
    t_in = nc.declare_dram_parameter("t", [rows, V], F32, isOutput=False)
    s_in = nc.declare_dram_parameter("s", [rows, V], F32, isOutput=False)
    o_out = nc.declare_dram_parameter("o", [P, OCOLS * nt], F32, isOutput=True)

    with TileContext(nc) as tc:
        with (
            tc.tile_pool(name="tea", bufs=3) as tea,
            tc.tile_pool(name="stu", bufs=3) as stu,
            tc.tile_pool(name="pck", bufs=2) as pck,
            tc.tile_pool(name="cnd", bufs=2) as cnd,
            tc.tile_pool(name="singles", bufs=1) as singles,
        ):
            out_t = singles.tile([P, OCOLS * nt], F32)
            dump_a = singles.tile([P, SC], BF16)   # ACT exp dump

            for it in range(nt):
                r0 = it * P
                ob = OCOLS * it
                cand = cnd.tile([P, NCAND], F32, tag="cand")

                for u in range(NCH):
                    a = tea.tile([P, SC], F32, tag="a")
                    s = stu.tile([P, SC], F32, tag="s")
                    # two HWDGE queues: teacher via sync, student via ACT
                    nc.sync.dma_start(
                        out=a, in_=t_in[r0:r0 + P, u * SC:(u + 1) * SC])
                    nc.scalar.dma_start(
                        out=s, in_=s_in[r0:r0 + P, u * SC:(u + 1) * SC])

                    # S_t / S_s partial sums (free accum on the exp passes)
                    nc.scalar.activation(
                        out=dump_a, in_=a, func=ACTF.Exp,
                        accum_out=out_t[:, ob + K + u:ob + K + u + 1],
                    )
                    nc.scalar.activation(
                        out=dump_a, in_=s, func=ACTF.Exp,
                        accum_out=out_t[:, ob + K + NCH + u:ob + K + NCH + u + 1],
                    )

                    # pack: high fp16 lanes <- t, low fp16 lanes <- s
                    up = pck.tile([P, SC], F32, tag="u", name=f"u{it}_{u}")
                    uph = up[:, :].bitcast(FP16)
                    nc.vector.tensor_copy(uph[:, 1::2], a)
                    nc.vector.tensor_copy(uph[:, 0::2], s)

                    # per-block top-8 of packed pairs
                    for b in range(BPC):
                        g = u * BPC + b
                        nc.vector.max(
                            out=cand[:, g * 8:(g + 1) * 8],
                            in_=up[:, b * BLK:(b + 1) * BLK],
                        )

                # 4 rounds -> top-32 packed pairs, written straight to out
                for r in range(4):
                    nc.vector.max(
                        out=out_t[:, ob + r * 8:ob + (r + 1) * 8], in_=cand)
                    if r < 3:
                        nc.vector.match_replace(
                            out=cand,
                            in_to_replace=out_t[:, ob + r * 8:ob + (r + 1) * 8],
                            in_values=cand, imm_value=NEG,
                        )

            nc.sync.dma_start(out=o_out[:, :], in_=out_t[:, :])

    nc.finalize()
    return nc


_NC_CACHE = None


def _get_nc():
    global _NC_CACHE
    if _NC_CACHE is None:
        _NC_CACHE = build_nc()
    return _NC_CACHE


def run_device(t2d, s2d, trace=False):
    """t2d/s2d: (N, V) float32. Returns BassKernelResults."""
    nc = _get_nc()
    in_maps = []
    for c in range(NCORES):
        sl = slice(c * ROWS, (c + 1) * ROWS)
        in_maps.append({
            "t": np.ascontiguousarray(t2d[sl]),
            "s": np.ascontiguousarray(s2d[sl]),
        })
    return run_bass_kernel_spmd(nc, in_maps, list(range(NCORES)), trace=trace)


def kernel(student_logits, teacher_logits):
    s2d = np.asarray(student_logits, dtype=np.float32).reshape(N, V)
    t2d = np.asarray(teacher_logits, dtype=np.float32).reshape(N, V)
    res = run_device(t2d, s2d)

    # gather per-row quantities: packed top-32 pairs + S_t/S_s partials
    p32 = np.empty((N, K), dtype=np.uint32)
    s_t = np.empty(N, dtype=np.float64)
    s_s = np.empty(N, dtype=np.float64)
    for c in range(NCORES):
        o = np.asarray(res.results[c]["o"])  # [P, OCOLS*NT] f32
        ob = o.view(np.uint32)
        for it in range(NT):
            r = slice(c * ROWS + it * P, c * ROWS + (it + 1) * P)
            col = OCOLS * it
            p32[r] = ob[:, col:col + K]
            s_t[r] = o[:, col + K:col + K + NCH].astype(np.float64).sum(1)
            s_s[r] = o[:, col + K + NCH:col + K + 2 * NCH].astype(
                np.float64).sum(1)

    # unpack fp16 halves -> t32, s32 (f64)
    t32 = (p32 >> 16).astype(np.uint16).view(np.float16).astype(np.float64)
    s32 = (p32 & 0xFFFF).astype(np.uint16).view(np.float16).astype(np.float64)

    # host finals in f64, replicating the reference on this support
    a_t = np.exp(t32).sum(1)
    p_t = a_t / s_t
    p_s = np.exp(s32).sum(1) / s_s

    log_ps = np.maximum(np.log(p_s), -100.0)
    log_1mps = np.maximum(np.log1p(-p_s), -100.0)
    loss_b = np.mean(-(p_t * log_ps + (1.0 - p_t) * log_1mps))

    th = t32 / 2.0
    sh = s32 / 2.0
    log_p = th - (np.log(np.exp(th - th.max(1, keepdims=True)).sum(1))
                  + th.max(1)).reshape(-1, 1)
    log_q = sh - (np.log(np.exp(sh - sh.max(1, keepdims=True)).sum(1))
                  + sh.max(1)).reshape(-1, 1)
    p = np.exp(log_p)
    loss_t = (p * (log_p - log_q)).sum(1).mean()

    return np.float32(loss_b + p_t.mean() * 4.0 * loss_t)
